# revision 9
# baseline (speedup 1.0000x reference)
"""Trainium2 Bass kernel for 2-layer residual BiLSTM (B=256, T=512, D=U=256).

Strategy v3 (direction-split data parallel + on-core layer pipelining):
  - Cores 0-3 run the FORWARD direction on batch quarters (64 rows each);
    cores 4-7 run BACKWARD on time-reversed inputs (same SPMD program --
    reversal happens host-side, outputs un-reversed on assembly). The fw and
    bw chains only meet at the final average, which the host computes.
  - Each core runs BOTH layers as two interleaved streams: layer 1 lags
    layer 0 by LAG steps and consumes h0 from an SBUF ring (no DRAM
    round-trip, no phase barrier). Layer-1 residual (h1 + h0) reads the same
    ring.
  - Doubling the per-stream batch to 64 (vs 32 in the data-parallel layout)
    doubles the moving columns per recurrent matmul, halving the
    weight-load-bound PE cost, and halves the ACT instruction count per
    step (the per-instruction overhead ~185ns rivals the payload).
  - "T-layout": gate/unit dims on partitions, batch on the free dim; z =
    Wx^T x (+ones bias via ACT) chunked TCP steps at a time into PSUM, with
    recurrent matmuls accumulating on top. Gate column order [g,i,f,o] so
    tanh(g) is one ACT and sigmoid(i,f,o) is one packed N=384 ACT.
  - Weights / x / h in fp16 (PE 1 cyc/row, FWL weight loads), PSUM/c fp32.
"""

import os

os.environ.setdefault("JAX_COMPILATION_CACHE_DIR", "/tmp/bilstm_jax_cache")

import numpy as np

# Problem shape (hardcoded per harness contract)
B, T, D, U = 256, 512, 256, 256
NCORES = 8
BS = B // (NCORES // 2)  # 64 batch rows per core (one direction per core)
G4 = 4 * U               # 1024 gate columns
NM = G4 // 128           # 8 m-chunks of gate columns
NK = U // 128            # 2 k-chunks of contraction dim
TCP = 4                  # steps per PSUM projection chunk (4 banks / layer)
TCX = 32                 # steps per input ring chunk
LAG = 10                 # layer-1 lag (== 2 mod TCP staggers psum refills)
W = 16                   # h0 SBUF ring slots (multiple of TCP, > LAG + TCP)

# gate column permutation: original order [i f g o] -> ours [g i f o]
_GATE_PERM = np.r_[2 * U:3 * U, 0:U, U:2 * U, 3 * U:4 * U]

_BUILD_CACHE = {}


def _build(T_, dtype="fp16", steps=None, out_slots=None):
    """Build the SPMD Bass program (same program on all cores).

    steps/out_slots: timing-only variants — run fewer scan steps and/or
    write outputs into a small ring (out[t % out_slots]) to shrink the
    per-call output volume. Defaults (None) build the real kernel.
    """
    from contextlib import ExitStack

    import concourse.bacc as bacc
    import concourse.bass as bass
    import concourse.mybir as mybir
    import concourse.tile as tile

    steps = T_ if steps is None else steps
    oslots = T_ if out_slots is None else out_slots

    f32 = mybir.dt.float32
    wdt = {"fp32": f32, "bf16": mybir.dt.bfloat16, "fp16": mybir.dt.float16}[dtype]
    AF = mybir.ActivationFunctionType

    nc = bacc.Bacc("TRN2", target_bir_lowering=False, debug=False)

    xT = nc.dram_tensor("xT", [NK, 128, T_, BS], wdt, kind="ExternalInput")
    Wd = {}
    for l in (0, 1):
        for wch in "xh":
            Wd[l, wch] = nc.dram_tensor(
                f"W{wch}{l}", [NK, 128, G4], wdt, kind="ExternalInput"
            )
    out_t = nc.dram_tensor("out", [oslots, 128, NK, BS], f32, kind="ExternalOutput")

    with ExitStack() as ctx:
        tc = ctx.enter_context(tile.TileContext(nc))
        wpool = ctx.enter_context(tc.tile_pool(name="w", bufs=1))
        ring = ctx.enter_context(tc.tile_pool(name="ring", bufs=3))
        state = ctx.enter_context(tc.tile_pool(name="state", bufs=1))
        gates = ctx.enter_context(tc.tile_pool(name="gates", bufs=3))
        outp = ctx.enter_context(tc.tile_pool(name="outp", bufs=6))
        psum = ctx.enter_context(
            tc.tile_pool(name="psum", bufs=1, space=bass.MemorySpace.PSUM)
        )

        # --- load weights (both layers) into SBUF once ---
        wsb = {}
        for l in (0, 1):
            for wch in "xh":
                t = wpool.tile([128, NK, G4], wdt, tag=f"W{wch}{l}", name=f"W{wch}{l}sb")
                for k in range(NK):
                    nc.sync.dma_start(t[:, k, :], Wd[l, wch][k])
                wsb[l, wch] = t

        # persistent state
        h0r = state.tile([128, NK, W, BS], wdt, tag="h0r", name="h0r")  # L0 h ring
        c0 = state.tile([128, NK, BS], f32, tag="c0", name="c0")
        c1 = state.tile([128, NK, BS], f32, tag="c1", name="c1")
        h1 = [
            state.tile([128, NK, BS], wdt, tag=f"h1_{i}", name=f"h1_{i}")
            for i in range(4)
        ]
        nc.gpsimd.memset(h0r[:, :, W - 1, :], 0.0)
        nc.gpsimd.memset(c0[:], 0.0)
        nc.gpsimd.memset(c1[:], 0.0)
        nc.gpsimd.memset(h1[0][:], 0.0)

        ringt = None   # (tile, base_t) for L0 x ring
        z0 = z1 = None
        hidx1 = 0

        def chain_a(sfx, z, j, c):
            """Gate ACTs + cell update for one layer-step. Split from
            chain_b so the two layers' chain instructions interleave on the
            ACT/DVE FIFOs at half-chain granularity (emitting a layer's
            tanh(c) before the other layer's gate ACTs head-of-line blocks
            the strict-FIFO engines)."""
            tg = gates.tile([128, NK, BS], wdt, tag=f"tg{sfx}")
            nc.scalar.activation(tg[:], z[:, 0:NK, j, :], AF.Tanh, bias=1.0)
            sif = gates.tile([128, 3 * NK, BS], wdt, tag=f"sif{sfx}")
            nc.scalar.activation(sif[:], z[:, NK:NM, j, :], AF.Sigmoid, bias=1.0)
            ig = gates.tile([128, NK, BS], wdt, tag=f"ig{sfx}")
            nc.vector.tensor_mul(ig[:], sif[:, 0:NK, :], tg[:])
            fc = gates.tile([128, NK, BS], f32, tag=f"fc{sfx}")
            nc.vector.tensor_mul(fc[:], sif[:, NK:2 * NK, :], c[:])
            nc.vector.tensor_add(c[:], ig[:], fc[:])
            return sif

        def chain_b(sfx, c, sif, hout):
            th = gates.tile([128, NK, BS], wdt, tag=f"th{sfx}")
            nc.scalar.activation(th[:], c[:], AF.Tanh)
            nc.vector.tensor_mul(hout, sif[:, 2 * NK:3 * NK, :], th[:])

        for r in range(steps + LAG):
            # ---------------- layer 0 at t0 = r ----------------
            if r < steps:
                t0 = r
                if t0 % TCX == 0:
                    rt = ring.tile([128, NK, TCX, BS], wdt, tag="ring0")
                    for k in range(NK):
                        nc.sync.dma_start(rt[:, k, :, :], xT[k, :, t0:t0 + TCX, :])
                    ringt = (rt, t0)
                rt, tb = ringt

                if t0 % TCP == 0:
                    z0 = psum.tile([128, NM, TCP, BS], f32, tag="z0")
                    for m in range(NM):
                        for k in range(NK):
                            nc.tensor.matmul(
                                z0[:, m, :, :],
                                wsb[0, "x"][:, k, m * 128:(m + 1) * 128],
                                rt[:, k, t0 - tb:t0 - tb + TCP, :],
                                start=(k == 0 and m % 2 == 0),
                                stop=False,
                                skip_group_check=True,
                            )
                j0 = t0 % TCP
                for m in range(NM):
                    for k in range(NK):
                        nc.tensor.matmul(
                            z0[:, m, j0, :],
                            wsb[0, "h"][:, k, m * 128:(m + 1) * 128],
                            h0r[:, k, (t0 - 1) % W, :],
                            start=False,
                            stop=(j0 == TCP - 1 and k == NK - 1 and m % 2 == 1),
                            skip_group_check=True,
                        )

            # ---------------- layer 1 at t1 = r - LAG ----------------
            if r >= LAG:
                t1 = r - LAG
                if t1 % TCP == 0:
                    s = t1 % W
                    z1 = psum.tile([128, NM, TCP, BS], f32, tag="z1")
                    for m in range(NM):
                        for k in range(NK):
                            nc.tensor.matmul(
                                z1[:, m, :, :],
                                wsb[1, "x"][:, k, m * 128:(m + 1) * 128],
                                h0r[:, k, s:s + TCP, :],
                                start=(k == 0 and m % 2 == 0),
                                stop=False,
                                skip_group_check=True,
                            )
                j1 = t1 % TCP
                for m in range(NM):
                    for k in range(NK):
                        nc.tensor.matmul(
                            z1[:, m, j1, :],
                            wsb[1, "h"][:, k, m * 128:(m + 1) * 128],
                            h1[hidx1 % 4][:, k, :],
                            start=False,
                            stop=(j1 == TCP - 1 and k == NK - 1 and m % 2 == 1),
                            skip_group_check=True,
                        )

            # ---------------- gate chains (phase-interleaved) ----------------
            sif0 = sif1 = None
            if r < steps:
                t0 = r
                sif0 = chain_a("0", z0, t0 % TCP, c0)
            if r >= LAG:
                t1 = r - LAG
                sif1 = chain_a("1", z1, t1 % TCP, c1)
            if sif0 is not None:
                chain_b("0", c0, sif0, h0r[:, :, t0 % W, :])
            if sif1 is not None:
                hn = h1[(hidx1 + 1) % 4]
                chain_b("1", c1, sif1, hn[:])
                # residual + output staging: accumulate TCP steps per DMA
                jo = t1 % TCP
                if jo == 0:
                    otile = outp.tile([128, TCP, NK, BS], f32, tag="ot")
                nc.gpsimd.tensor_add(
                    otile[:, jo, :, :], hn[:], h0r[:, :, t1 % W, :]
                )
                if jo == TCP - 1:
                    t1b = (t1 - (TCP - 1)) % oslots
                    nc.sync.dma_start(
                        out_t[t1b:t1b + TCP].rearrange("t p k b -> p t (k b)"),
                        otile.rearrange("p t k b -> p t (k b)"),
                    )
                hidx1 += 1

    nc.compile()
    return nc


def _prep_inputs(inputs, T_, dtype="fp16"):
    """Host-side shard + layout prep. Returns per-core input maps."""
    import ml_dtypes

    wdt = {"fp32": np.float32, "bf16": ml_dtypes.bfloat16, "fp16": np.float16}[dtype]

    x = np.asarray(inputs["x"], dtype=np.float32)

    wmaps = {}  # per direction
    for d, dd in (("f", "fw"), ("b", "bw")):
        m = {}
        for l in (0, 1):
            for wch, key in (("x", "Wx"), ("h", "Wh")):
                w = np.asarray(inputs[f"{dd}{l}_{key}"], dtype=np.float32)
                wp = w[:, _GATE_PERM].reshape(NK, 128, G4)
                m[f"W{wch}{l}"] = np.ascontiguousarray(wp).astype(wdt)
            bb = np.asarray(inputs[f"{dd}{l}_b"], dtype=np.float32)
            if not np.allclose(bb, 1.0, atol=0.0):
                raise NotImplementedError(
                    "kernel assumes bias == ones (keras bias_initializer='ones')"
                )
        wmaps[d] = m

    in_maps = []
    for ci in range(NCORES):
        d = "f" if ci < 4 else "b"
        q = ci % 4
        xs = x[q * BS:(q + 1) * BS, :T_, :]            # [BS, T_, D]
        if d == "b":
            xs = xs[:, ::-1, :]                        # time-reverse for bw
        xTc = np.ascontiguousarray(xs.transpose(2, 1, 0))  # [D, T_, BS]
        xTc = xTc.reshape(NK, 128, T_, BS).astype(wdt)
        m = {"xT": xTc}
        m.update(wmaps[d])
        in_maps.append(m)
    return in_maps


def _assemble(results, T_):
    out = np.empty((B, T_, U), dtype=np.float32)
    for q in range(4):
        af = results[q]["out"]          # [T_, 128, NK, BS] fw
        ab = results[q + 4]["out"]      # [T_, 128, NK, BS] bw (reversed time)
        # out[b, t, k*128 + p] = arr[t, p, k, b]
        f = af.transpose(3, 0, 2, 1).reshape(BS, T_, U)
        bwd = ab[::-1].transpose(3, 0, 2, 1).reshape(BS, T_, U)
        out[q * BS:(q + 1) * BS] = (f + bwd) * 0.5
    return out


def _setup_jax_cache():
    try:
        import jax

        jax.config.update("jax_compilation_cache_dir",
                          os.environ["JAX_COMPILATION_CACHE_DIR"])
        jax.config.update("jax_persistent_cache_min_compile_time_secs", 1.0)
        jax.config.update("jax_persistent_cache_min_entry_size_bytes", 0)
    except Exception:
        pass


def kernel(**inputs) -> np.ndarray:
    _setup_jax_cache()
    from concourse.bass_utils import run_bass_kernel_spmd

    dtype = "fp16"
    key = (T, dtype)
    if key not in _BUILD_CACHE:
        _BUILD_CACHE[key] = _build(T, dtype)
    nc = _BUILD_CACHE[key]

    in_maps = _prep_inputs(inputs, T, dtype)
    res = run_bass_kernel_spmd(nc, in_maps, core_ids=list(range(NCORES)))
    return _assemble(res.results, T)
